# revision 11
# baseline (speedup 1.0000x reference)
"""Trainium2 Bass kernel for EfmLSTM (signature-gated LSTM), 8-core data-parallel.

Strategy
--------
Data-parallel over batch: B=64 -> 8 cores x B_loc=8. Everything on-chip uses a
"units-on-partition" transposed layout so the sequential scan needs no
transposes at all:

  h^T, c^T, f^T, gate tensors are [128 partitions, u*8+b] where unit = 128*u+p.

Per timestep (per core):
  gates^T: 12 chunks of (gate_type, unit_chunk) x 8 batch =
    sum_k W_rec[k-chunk, m-chunk]-stationary @ h^T[:, k-chunk]  (48 bf16
    matmuls, N=8 moving) accumulated in 3 PSUM banks (one per gate group so
    the c~/i elementwise chains overlap the o matmuls), + x^T_t via DVE,
  then ACT sigmoid/tanh on [128, 32] slices, DVE for the c/h updates.

x^T = inputs @ W_in and f^T = sigmoid(signatures @ W_f + b_f) are precomputed
per 128-step chunk with full-width matmuls (cheap), biases folded in at the
PSUM->SBUF eviction.

The T loop is a hardware For_i over chunk PAIRS (ping-pong SBUF slots inside
the body) — keeps the program ~16K instructions instead of ~60K.

Host side pre-permutes weights into gate order [c~, i, o] and pre-transposes /
pre-casts inputs to bf16, so the device never transposes anything.
"""

import numpy as np
import ml_dtypes

# Problem shapes (hardcoded per harness contract)
B, T, F = 64, 1024, 256
U = 512
SIG = 31
NCORES = 8
BL = B // NCORES  # 8 batch per core

T_CHUNK = 128
KC = U // 128        # 4 k-chunks of h/units
MC = (3 * U) // 128  # 12 gate-column chunks
FC = F // 128        # 2 k-chunks of input features

_cache = {}


def _split_excess_waits(nc, limit=1):
    """This walrus build rejects >1 sync-wait command per instruction
    ('Too many sync wait commands', CoreV2/V3 setupSyncWait). Hoist excess
    waits onto same-engine NoOp instructions inserted just before the
    offending instruction — the engine sequencer processes its queue in
    order, so the waits still complete before the instruction issues."""
    import concourse.mybir as mybir
    import bass_rust as _br

    cnt = 0
    for f in nc.m.functions:
        for b in f.blocks:
            il = b.instructions
            if not any(
                i.sync_info and i.sync_info.on_wait and len(i.sync_info.on_wait) > limit
                for i in il
            ):
                continue
            new = []
            for inst in il:
                si = inst.sync_info
                waits = list(si.on_wait) if si and si.on_wait else []
                if len(waits) > limit:
                    for w in waits[:-limit]:
                        nop = mybir.InstNoOp(name=f"wsplit_{cnt}", ins=[], outs=[])
                        cnt += 1
                        nop.engine = inst.engine
                        nop.sync_info = _br.SyncInfo(on_wait=[w], on_update=[])
                        new.append(nop)
                    si.on_wait = waits[-limit:]
                new.append(inst)
            il[:] = new
    return cnt


WREC_SCALE = 16.0  # wrec pre-scaled x16 into fp8e3m4's normal range


def _ws_for(wrec_fp8, fused):
    # fused path halves/quarters wrec on the host (sigmoid-as-tanh column
    # scales + h~=2h row scale), so a larger fp8 scale keeps values normal
    return (64.0 if fused else WREC_SCALE) if wrec_fp8 else 1.0


def _build_nc(compute_dt_name="bfloat16", wrec_dt_name="bfloat16", t_eff=T,
              shadow_prewrite=False, o_split=False, pre_spread=False,
              x_via_mm=False, diag_no_h=False, act_order_b=False,
              fused_chain=False):
    import concourse.bass as bass
    import concourse.mybir as mybir
    import concourse.tile as tile
    from contextlib import ExitStack

    fp32 = mybir.dt.float32
    cdt = getattr(mybir.dt, compute_dt_name)
    wdt = getattr(mybir.dt, wrec_dt_name)
    ws = _ws_for(wrec_dt_name.startswith("float8"), fused_chain)
    AF = mybir.ActivationFunctionType
    ALU = mybir.AluOpType
    ds = bass.ds

    assert t_eff % (2 * T_CHUNK) == 0
    nc = bass.Bass()

    # t dim padded by one body (2*T_CHUNK) so the loop's next-iteration
    # prefetch never reads out of bounds
    t_pad = t_eff + 2 * T_CHUNK
    x_in = nc.declare_dram_parameter("inputs_t", [F, BL, t_pad], cdt, isOutput=False)
    sig_in = nc.declare_dram_parameter("sig_t", [SIG, BL, t_pad], cdt, isOutput=False)
    wrec_in = nc.declare_dram_parameter("wrec", [128, KC * MC * 128], wdt, isOutput=False)
    win_in = nc.declare_dram_parameter("win", [128, FC * MC * 128], cdt, isOutput=False)
    wsig_in = nc.declare_dram_parameter("wsig", [SIG, U], cdt, isOutput=False)
    bias_g_in = nc.declare_dram_parameter("bias_g", [128, MC], fp32, isOutput=False)
    bias_f_in = nc.declare_dram_parameter("bias_f", [128, KC], fp32, isOutput=False)
    ident_in = (nc.declare_dram_parameter("ident", [128, 128], cdt, isOutput=False)
                if x_via_mm else None)
    h_out = nc.declare_dram_parameter("h_out", [128, KC * BL], fp32, isOutput=True)

    with ExitStack() as ctx:
        tc = ctx.enter_context(tile.TileContext(nc))

        const = ctx.enter_context(tc.tile_pool(name="const", bufs=1))
        state = ctx.enter_context(tc.tile_pool(name="state", bufs=1))
        data = ctx.enter_context(tc.tile_pool(name="data", bufs=1))
        work = ctx.enter_context(tc.tile_pool(name="work", bufs=6 if diag_no_h else 3))
        if fused_chain:
            psum_s = ctx.enter_context(tc.tile_pool(name="psum_s", bufs=1, space="PSUM"))
        else:
            psum_g = ctx.enter_context(tc.tile_pool(name="psum_g", bufs=2, space="PSUM"))
        psum_p = ctx.enter_context(tc.tile_pool(name="psum_p", bufs=2, space="PSUM"))

        wrec = const.tile([128, KC * MC * 128], wdt)
        nc.sync.dma_start(out=wrec[:], in_=wrec_in[:])
        win = const.tile([128, FC * MC * 128], cdt)
        nc.sync.dma_start(out=win[:], in_=win_in[:])
        wsig = const.tile([SIG, U], cdt)
        nc.sync.dma_start(out=wsig[:], in_=wsig_in[:])
        bias_g = const.tile([128, MC], fp32)
        nc.sync.dma_start(out=bias_g[:], in_=bias_g_in[:])
        bias_f = const.tile([128, KC], fp32)
        nc.sync.dma_start(out=bias_f[:], in_=bias_f_in[:])
        if x_via_mm:
            ident = const.tile([128, 128], cdt)
            nc.sync.dma_start(out=ident[:], in_=ident_in[:])

        h_bf = state.tile([128, KC * BL], cdt)      # h^T bf16, col = 8*k + b
        c_st = state.tile([128, KC * BL], fp32)     # c^T fp32
        nc.vector.memset(h_bf[:], 0.0)
        nc.vector.memset(c_st[:], 0.0)
        if diag_no_h:
            # DIAGNOSTIC ONLY (wrong results): matmuls read this constant
            # instead of h, removing the DVE->PE h dependency to measure the
            # pure LDW-stream floor.
            h_const = state.tile([128, KC * BL], cdt)
            nc.vector.memset(h_const[:], 0.01)

        W = KC * BL     # 32

        if fused_chain:
            # Fused-tanh chain: sigma(x) = (tanh(x/2)+1)/2 with the /2 folded
            # into the host-side i/o weight columns, so ALL gate activations
            # are Tanh and fuse into 2 ACT ops per step:
            #   P_ci [128, 2W] holds [c~ | i] preacts  -> ACT_a = tanh(P_ci)
            #   P_oc [128, 2W] holds [o^ | c_state]    -> ACT_b = tanh(P_oc)
            # c lives IN PSUM next to o^ so one ACT covers both; h~ = 2h =
            # (tanh(o^)+1) * tanh(c) feeds the matmuls with W_rec rows halved.
            p_ci = psum_s.tile([128, 2 * W], fp32, name="pci")
            p_oc = psum_s.tile([128, 2 * W], fp32, name="poc")
            scr = work.tile([128, 2 * W], cdt, tag="scr", name="scr")
            nc.vector.memset(scr[:], 0.0)
            # warm has_written once on the matmul-accumulated regions
            nc.tensor.matmul(p_ci[:], lhsT=wrec[:, 0:128], rhs=scr[:],
                             start=True, stop=True)
            nc.tensor.matmul(p_oc[:, 0:W], lhsT=wrec[:, 0:128], rhs=scr[:, 0:W],
                             start=True, stop=True)
            nc.vector.memset(p_oc[:, W:2 * W], 0.0)  # c0 = 0

        # Warm up the 6 scan PSUM bank slots (3 gate groups x 2 bufs) with a
        # dummy start=True matmul each: this sets every element's has_written
        # bit once and we never clear it again. From then on the scan
        # pre-writes x^T into the bank via DVE and the recurrent matmuls
        # accumulate on top with start=False (a DVE write does not clear
        # has_written - only a start=True matmul does).
        # (Not needed when x is injected via identity matmul with start=True.)
        if not x_via_mm and not fused_chain:
            for warm in range(2):
                for gi in range(3):
                    pg_t = psum_g.tile([128, KC * BL], fp32, tag=f"pg{gi}", name=f"pg{gi}")
                    for jj in range(KC):
                        nc.tensor.matmul(
                            pg_t[:, jj * BL:(jj + 1) * BL],
                            lhsT=wrec[:, jj * 128:(jj + 1) * 128],
                            rhs=h_bf[:, 0:BL],
                            start=True, stop=True,
                        )

        srcv = x_in.rearrange("(k p) b t -> p k b t", p=128)

        # per-half static tile sets (ping-pong inside the For_i body)
        halves = []
        for hf in range(2):
            in_sb = data.tile([128, FC * BL * T_CHUNK], cdt, name=f"in_sb{hf}")
            sig_sb = data.tile([SIG, BL * T_CHUNK], cdt, name=f"sig_sb{hf}")
            x_sb = data.tile([128, T_CHUNK * MC * BL], cdt, name=f"x_sb{hf}")
            f_sb = data.tile([128, T_CHUNK * KC * BL], cdt, name=f"f_sb{hf}")
            halves.append((in_sb, sig_sb, x_sb, f_sb))

        NT = 512 // BL  # timesteps covered per 512-wide matmul
        W = KC * BL     # 32

        def emit_dmas(t0sc, hf):
            """Stage the half's inputs+signatures from DRAM."""
            in_sb, sig_sb, x_sb, f_sb = halves[hf]
            in_sbv = in_sb.rearrange("p (k b t) -> p k b t", k=FC, b=BL)
            for k in range(FC):
                nc.sync.dma_start(out=in_sbv[:, k, :, :],
                                  in_=srcv[:, k, :, ds(t0sc, T_CHUNK)])
            nc.sync.dma_start(out=sig_sb.rearrange("p (b t) -> p b t", b=BL),
                              in_=sig_in[:, :, ds(t0sc, T_CHUNK)])

        def pre_groups(hf):
            """Generator: one x/f precompute group (matmuls + eviction) per
            next() — lets the scan interleave these into its PE/ACT idle
            windows."""
            in_sb, sig_sb, x_sb, f_sb = halves[hf]
            in_sb4 = in_sb.rearrange("p (k b t) -> p k b t", k=FC, b=BL)
            x_sb4 = x_sb.rearrange("p (t m b) -> p m b t", m=MC, b=BL)
            f_sb4 = f_sb.rearrange("p (t u b) -> p u b t", u=KC, b=BL)
            sig_sb3 = sig_sb.rearrange("p (b t) -> p b t", b=BL)
            for j in range(MC):
                for th in range(T_CHUNK // NT):
                    ps = psum_p.tile([128, 512], fp32, tag="ps_pre", name="ps_pre")
                    for k in range(FC):
                        nc.tensor.matmul(
                            ps[:],
                            lhsT=win[:, (k * MC + j) * 128:(k * MC + j + 1) * 128],
                            rhs=in_sb4[:, k, :, th * NT:(th + 1) * NT],
                            start=(k == 0), stop=(k == FC - 1),
                        )
                        # finer-grained interleave: one ~400ns matmul per
                        # yield fits the per-step PE idle window
                        if pre_spread and k < FC - 1:
                            yield
                    dst = x_sb4[:, j, :, th * NT:(th + 1) * NT]
                    # store ws*(x + b): the scan accumulates ws*W_rec@h on top
                    # and descales at the gate activations
                    nc.scalar.activation(
                        dst, ps[:], AF.Identity, bias=bias_g[:, j:j + 1],
                        scale=ws)
                    yield
            for u in range(KC):
                for th in range(T_CHUNK // NT):
                    ps = psum_p.tile([128, 512], fp32, tag="ps_pre", name="ps_pre")
                    nc.tensor.matmul(
                        ps[:],
                        lhsT=wsig[:, u * 128:(u + 1) * 128],
                        rhs=sig_sb3[:, :, th * NT:(th + 1) * NT],
                        start=True, stop=True,
                    )
                    dst = f_sb4[:, u, :, th * NT:(th + 1) * NT]
                    nc.scalar.activation(
                        dst, ps[:], AF.Sigmoid, bias=bias_f[:, u:u + 1])
                    yield

        def scan_fused(hf, pre_iter, x_next):
            """Fused-tanh scan body: 2 ACT ops per step (tanh over [c~|i],
            tanh over [o^|c]), c state lives in PSUM beside o^. Returns the
            last step's s_b tile ([tanh(o^) | tanh(c)])."""
            in_sb, sig_sb, x_sb, f_sb = halves[hf]
            h_src = h_const if diag_no_h else h_bf
            s_b = None

            def mmf(j_list):
                for j in j_list:
                    dst = (p_ci[:, j * BL:(j + 1) * BL] if j < 8
                           else p_oc[:, (j - 8) * BL:(j - 7) * BL])
                    for k in range(KC):
                        nc.tensor.matmul(
                            dst,
                            lhsT=wrec[:, (k * MC + j) * 128:(k * MC + j + 1) * 128],
                            rhs=h_src[:, k * BL:(k + 1) * BL],
                            start=False, stop=(k == KC - 1),
                            skip_group_check=True,
                        )

            for tt in range(T_CHUNK):
                # c = f*c in place in PSUM; waits only on last step's ACT_b read
                nc.vector.scalar_tensor_tensor(
                    p_oc[:, W:2 * W], f_sb[:, tt * W:(tt + 1) * W], 1.0,
                    p_oc[:, W:2 * W], op0=ALU.mult, op1=ALU.mult)
                mmf(range(0, 8))
                s_a = work.tile([128, 2 * W], fp32, tag="s_a", name="s_a")
                nc.scalar.activation(s_a[:], p_ci[:], AF.Tanh, scale=1.0 / ws)
                tmp = work.tile([128, W], fp32, tag="tmp", name="tmp")
                # 2*i*c~ = (tanh(a_i/2)+1) * tanh(a_c)
                nc.vector.scalar_tensor_tensor(
                    tmp[:], s_a[:, W:2 * W], 1.0, s_a[:, 0:W],
                    op0=ALU.add, op1=ALU.mult)
                # c += 0.5*(2 i c~)  (state kept ws-scaled for uniform ACT_b)
                nc.vector.scalar_tensor_tensor(
                    p_oc[:, W:2 * W], tmp[:], 0.5 * ws, p_oc[:, W:2 * W],
                    op0=ALU.mult, op1=ALU.add)
                # next step's [c~|i] prewrite, off the h critical path (only
                # needs this step's ACT_a to have read the bank)
                if tt + 1 < T_CHUNK:
                    xs_n = x_sb[:, (tt + 1) * MC * BL:(tt + 2) * MC * BL]
                else:
                    xs_n = x_next[:, 0:MC * BL]
                nc.vector.tensor_copy(p_ci[:], xs_n[:, 0:2 * W])
                mmf(range(8, MC))
                s_b = work.tile([128, 2 * W], fp32, tag="s_b", name="s_b")
                nc.scalar.activation(s_b[:], p_oc[:], AF.Tanh, scale=1.0 / ws)
                # h~ = (tanh(a_o/2)+1)*tanh(c) = 2h  (W_rec rows pre-halved)
                nc.vector.scalar_tensor_tensor(
                    h_bf[:], s_b[:, 0:W], 1.0, s_b[:, W:2 * W],
                    op0=ALU.add, op1=ALU.mult)
                nc.vector.tensor_copy(p_oc[:, 0:W], xs_n[:, 2 * W:3 * W])
                if pre_iter is not None:
                    if tt % 4 == 3:
                        next(pre_iter, None)
            if pre_iter is not None:
                for _ in pre_iter:
                    pass
            return s_b

        def psum_prewrite(xs):
            """Allocate the 3 gate PSUM banks and pre-write x^T into them (the
            matmuls accumulate on top; has_written bits are permanently set,
            see warmup)."""
            pgs = []
            for gi in range(3):
                pg_t = psum_g.tile([128, W], fp32, tag=f"pg{gi}", name=f"pg{gi}")
                nc.vector.tensor_copy(pg_t[:], xs[:, gi * W:(gi + 1) * W])
                pgs.append(pg_t)
            return pgs

        def scan(hf, pre_iter, pgs, x_next):
            """pgs: this half's step-0 PSUM banks, already pre-written.
            x_next: x_sb of the half that runs after this one. Returns
            (s_o, tc, pgs) with pgs pre-written for that next half's step 0."""
            in_sb, sig_sb, x_sb, f_sb = halves[hf]
            s_o = tc_t = None

            h_src = h_const if diag_no_h else h_bf

            def mm(j_list, pgs):
                for j in j_list:
                    gi, jj = j // 4, j % 4
                    for k in range(KC):
                        nc.tensor.matmul(
                            pgs[gi][:, jj * BL:(jj + 1) * BL],
                            lhsT=wrec[:, (k * MC + j) * 128:(k * MC + j + 1) * 128],
                            rhs=h_src[:, k * BL:(k + 1) * BL],
                            start=False, stop=(k == KC - 1),
                            skip_group_check=True,
                        )

            for tt in range(T_CHUNK):
                xs = x_sb[:, tt * MC * BL:(tt + 1) * MC * BL]
                if x_via_mm:
                    # inject x^T into the banks with identity matmuls
                    # (start=True): pure PE work with NO h dependency, so the
                    # PE does it in the tail window while waiting for h, and
                    # the next step's matmuls wait only on the h semaphore.
                    pgs = []
                    for gi in range(3):
                        pg_t = psum_g.tile([128, W], fp32, tag=f"pg{gi}",
                                           name=f"pg{gi}")
                        nc.tensor.matmul(
                            pg_t[:], lhsT=ident[:], rhs=xs[:, gi * W:(gi + 1) * W],
                            start=True, stop=False, skip_group_check=True)
                        pgs.append(pg_t)
                elif not shadow_prewrite:
                    pgs = psum_prewrite(xs)
                # c = f*c can start as soon as the prior step's tanh(c) read it
                nc.vector.scalar_tensor_tensor(
                    c_st[:], f_sb[:, tt * W:(tt + 1) * W], 1.0, c_st[:],
                    op0=ALU.mult, op1=ALU.mult)
                acts = pgs
                if o_split:
                    # c~ and i matmuls; their activation chain runs under the
                    # o matmuls, which are split so sigma(o)/h for h-chunks
                    # k0/k1 complete before the last matmul -> next step's
                    # k0/k1 matmuls overlap the k2/k3 tail.
                    mm(range(0, 8), pgs)
                    s_cc = work.tile([128, W], fp32, tag="s_cc", name="s_cc")
                    nc.scalar.activation(s_cc[:], acts[0][:], AF.Tanh, scale=1.0 / ws)
                    s_i = work.tile([128, W], fp32, tag="s_i", name="s_i")
                    nc.scalar.activation(s_i[:], acts[1][:], AF.Sigmoid, scale=1.0 / ws)
                    tmp = work.tile([128, W], fp32, tag="tmp", name="tmp")
                    nc.vector.scalar_tensor_tensor(
                        tmp[:], s_i[:], 1.0, s_cc[:], op0=ALU.mult, op1=ALU.mult)
                    nc.vector.scalar_tensor_tensor(
                        c_st[:], c_st[:], 1.0, tmp[:], op0=ALU.mult, op1=ALU.add)
                    tc_t = work.tile([128, W], fp32, tag="tc", name="tc")
                    nc.scalar.activation(tc_t[:], c_st[:], AF.Tanh)
                    s_o = work.tile([128, W], fp32, tag="s_o", name="s_o")
                    HW_ = W // 2
                    mm((8, 9), pgs)
                    nc.scalar.activation(s_o[:, 0:HW_], acts[2][:, 0:HW_],
                                         AF.Sigmoid, scale=1.0 / ws)
                    nc.vector.scalar_tensor_tensor(
                        h_bf[:, 0:HW_], s_o[:, 0:HW_], 1.0, tc_t[:, 0:HW_],
                        op0=ALU.mult, op1=ALU.mult)
                    mm((10, 11), pgs)
                    nc.scalar.activation(s_o[:, HW_:W], acts[2][:, HW_:W],
                                         AF.Sigmoid, scale=1.0 / ws)
                    nc.vector.scalar_tensor_tensor(
                        h_bf[:, HW_:W], s_o[:, HW_:W], 1.0, tc_t[:, HW_:W],
                        op0=ALU.mult, op1=ALU.mult)
                else:
                    # 48 matmuls: m-outer (c~ 0-3, i 4-7, o 8-11), k-inner
                    mm(range(MC), pgs)
                    # pre-write the NEXT step's banks now, in the matmul
                    # shadow, off the h -> next-step-matmul critical path
                    if shadow_prewrite and tt + 1 < T_CHUNK:
                        pgs = psum_prewrite(
                            x_sb[:, (tt + 1) * MC * BL:(tt + 2) * MC * BL])
                    # activations straight from PSUM; sigma(o) is emitted
                    # before tanh(c) so ACT doesn't queue it behind the c chain
                    s_cc = work.tile([128, W], fp32, tag="s_cc", name="s_cc")
                    nc.scalar.activation(s_cc[:], acts[0][:], AF.Tanh, scale=1.0 / ws)
                    s_i = work.tile([128, W], fp32, tag="s_i", name="s_i")
                    nc.scalar.activation(s_i[:], acts[1][:], AF.Sigmoid, scale=1.0 / ws)
                    s_o = work.tile([128, W], fp32, tag="s_o", name="s_o")
                    if not act_order_b:
                        nc.scalar.activation(s_o[:], acts[2][:], AF.Sigmoid,
                                             scale=1.0 / ws)
                    tmp = work.tile([128, W], fp32, tag="tmp", name="tmp")
                    nc.vector.scalar_tensor_tensor(
                        tmp[:], s_i[:], 1.0, s_cc[:], op0=ALU.mult, op1=ALU.mult)
                    nc.vector.scalar_tensor_tensor(
                        c_st[:], c_st[:], 1.0, tmp[:], op0=ALU.mult, op1=ALU.add)
                    tc_t = work.tile([128, W], fp32, tag="tc", name="tc")
                    nc.scalar.activation(tc_t[:], c_st[:], AF.Tanh)
                    if act_order_b:
                        # tanh(c) ahead of sigma(o) in the ACT FIFO: the c
                        # chain isn't FIFO-gated on the o-group matmuls
                        nc.scalar.activation(s_o[:], acts[2][:], AF.Sigmoid,
                                             scale=1.0 / ws)
                    nc.vector.scalar_tensor_tensor(
                        h_bf[:], s_o[:], 1.0, tc_t[:], op0=ALU.mult, op1=ALU.mult)
                # fill the PE's h-wait idle window with precompute matmuls
                if pre_iter is not None:
                    if pre_spread:
                        if tt % 2 == 1:
                            next(pre_iter, None)
                    elif tt % 4 == 3:
                        next(pre_iter, None)
            if pre_iter is not None:
                for _ in pre_iter:
                    pass
            if shadow_prewrite:
                # next half's step-0 banks (x_sb complete: pre_iter drained)
                pgs = psum_prewrite(x_next[:, 0:MC * BL])
            return s_o, tc_t, pgs

        # prologue: half0 of the first iteration, serial
        emit_dmas(0, 0)
        for _ in pre_groups(0):
            pass

        if fused_chain:
            x0 = halves[0][2]
            nc.vector.tensor_copy(p_ci[:], x0[:, 0:2 * W])
            nc.vector.tensor_copy(p_oc[:, 0:W], x0[:, 2 * W:3 * W])
            with tc.For_i(0, t_eff, 2 * T_CHUNK) as t0:
                emit_dmas(t0 + T_CHUNK, 1)
                scan_fused(0, pre_groups(1), halves[1][2])
                emit_dmas(t0 + 2 * T_CHUNK, 0)
                s_b1 = scan_fused(1, pre_groups(0), halves[0][2])
            # h~ = 2h in fp32; the host multiplies the gathered output by 0.5
            h_f = state.tile([128, KC * BL], fp32)
            nc.vector.scalar_tensor_tensor(
                h_f[:], s_b1[:, 0:W], 1.0, s_b1[:, W:2 * W],
                op0=ALU.add, op1=ALU.mult)
            nc.sync.dma_start(out=h_out[:], in_=h_f[:])
        else:
            pgs = (psum_prewrite(halves[0][2][:, 0:MC * BL])
                   if shadow_prewrite else None)

            with tc.For_i(0, t_eff, 2 * T_CHUNK) as t0:
                emit_dmas(t0 + T_CHUNK, 1)           # this iteration's half1 data
                s_o0, tc0, pgs = scan(0, pre_groups(1), pgs, halves[1][2])
                emit_dmas(t0 + 2 * T_CHUNK, 0)       # next iteration's half0 data
                s_o1, tc1, pgs = scan(1, pre_groups(0), pgs, halves[0][2])

            # final h in fp32 from the last step's stashed (static-slot) tiles
            h_f = state.tile([128, KC * BL], fp32)
            nc.vector.scalar_tensor_tensor(
                h_f[:], s_o1[:], 1.0, tc1[:], op0=ALU.mult, op1=ALU.mult)
            nc.sync.dma_start(out=h_out[:], in_=h_f[:])

    _split_excess_waits(nc)
    return nc


def _prep_host_inputs(inputs, signatures, forget_kernel, input_kernel,
                      recurrent_kernel, bias, cdt=ml_dtypes.bfloat16, t_factor=1,
                      wrec_fp8=False, fused=False):
    """Host-side shard + permute + transpose + cast. Returns in_maps list."""
    # gate order in reference: [i, c~, o]; ours: [c~, i, o]
    perm = np.concatenate([np.arange(U, 2 * U), np.arange(0, U), np.arange(2 * U, 3 * U)])
    win_p = input_kernel[:, perm]          # [F, 3U]
    wrec_p = recurrent_kernel[:, perm]     # [U, 3U]
    b_i, b_f, b_c, b_o = np.split(bias, 4)
    bias_g = np.concatenate([b_c, b_i, b_o])  # per permuted gate col, [3U]

    if fused:
        # sigmoid-as-tanh: halve i/o preactivations (sigma(x)=(tanh(x/2)+1)/2)
        # and halve W_rec rows since the device h is h~ = 2h
        s_col = np.concatenate([np.ones(U), np.full(2 * U, 0.5)]).astype(np.float32)
        win_p = win_p * s_col[None, :]
        wrec_p = wrec_p * s_col[None, :] * 0.5
        bias_g = bias_g * s_col

    # wrec blocks: [128, (k*MC + j)*128 + c] = wrec_p[128*k + p, 128*j + c]
    wr = wrec_p.reshape(KC, 128, MC, 128).transpose(1, 0, 2, 3).reshape(128, KC * MC * 128)
    wi = win_p.reshape(FC, 128, MC, 128).transpose(1, 0, 2, 3).reshape(128, FC * MC * 128)
    bg = bias_g.reshape(MC, 128).T.copy()          # [128, MC]
    bf_ = b_f.reshape(KC, 128).T.copy()            # [128, KC]

    if wrec_fp8:
        ws = _ws_for(True, fused)
        wr = (wr * ws).astype(ml_dtypes.float8_e3m4)
        bg = bg * ws  # ACT bias is applied after the input scale
    else:
        wr = wr.astype(cdt)
    wi = wi.astype(cdt)
    wsig = forget_kernel.astype(cdt)               # [SIG, U]

    in_maps = []
    for c in range(NCORES):
        bsl = slice(c * BL, (c + 1) * BL)
        # [BL, T, F] -> [F, BL, T]
        x_t = np.ascontiguousarray(inputs[bsl].transpose(2, 0, 1)).astype(cdt)
        s_t = np.ascontiguousarray(signatures[bsl].transpose(2, 0, 1)).astype(cdt)
        if t_factor > 1:
            x_t = np.tile(x_t, (1, 1, t_factor))
            s_t = np.tile(s_t, (1, 1, t_factor))
        pad = 2 * T_CHUNK
        x_t = np.concatenate([x_t, np.zeros(x_t.shape[:2] + (pad,), x_t.dtype)], axis=2)
        s_t = np.concatenate([s_t, np.zeros(s_t.shape[:2] + (pad,), s_t.dtype)], axis=2)
        in_maps.append({
            "inputs_t": x_t, "sig_t": s_t, "wrec": wr, "win": wi,
            "wsig": wsig, "bias_g": bg.astype(np.float32),
            "bias_f": bf_.astype(np.float32),
            "ident": np.eye(128, dtype=cdt),
        })
    return in_maps


BEST_CONFIG = {"fused_chain": True}


def kernel(inputs, signatures, forget_kernel, input_kernel, recurrent_kernel,
           bias, _trace=False):
    inputs = np.asarray(inputs, dtype=np.float32)
    signatures = np.asarray(signatures, dtype=np.float32)
    forget_kernel = np.asarray(forget_kernel, dtype=np.float32)
    input_kernel = np.asarray(input_kernel, dtype=np.float32)
    recurrent_kernel = np.asarray(recurrent_kernel, dtype=np.float32)
    bias = np.asarray(bias, dtype=np.float32)

    from concourse.bass_utils import run_bass_kernel_spmd

    if "nc" not in _cache:
        _cache["nc"] = _build_nc(**BEST_CONFIG)
    nc = _cache["nc"]

    fused = BEST_CONFIG.get("fused_chain", False)
    wrec_fp8 = BEST_CONFIG.get("wrec_dt_name", "bfloat16").startswith("float8")
    in_maps = _prep_host_inputs(inputs, signatures, forget_kernel,
                                input_kernel, recurrent_kernel, bias,
                                wrec_fp8=wrec_fp8, fused=fused)
    res = run_bass_kernel_spmd(nc, in_maps, list(range(NCORES)), trace=_trace)

    out = np.empty((B, U), np.float32)
    for c in range(NCORES):
        hT = res.results[c]["h_out"]                  # [128, KC*BL]
        h = hT.reshape(128, KC, BL).transpose(2, 1, 0).reshape(BL, U)
        out[c * BL:(c + 1) * BL] = h
    if fused:
        out *= 0.5
    if _trace:
        return out, res
    return out



# revision 13
# speedup vs baseline: 1.2011x; 1.2011x over previous
"""Trainium2 Bass kernel for EfmLSTM (signature-gated LSTM), 8-core data-parallel.

Strategy
--------
Data-parallel over batch: B=64 -> 8 cores x B_loc=8. Everything on-chip uses a
"units-on-partition" transposed layout so the sequential scan needs no
transposes at all:

  h^T, c^T, f^T, gate tensors are [128 partitions, u*8+b] where unit = 128*u+p.

Per timestep (per core):
  gates^T: 12 chunks of (gate_type, unit_chunk) x 8 batch =
    sum_k W_rec[k-chunk, m-chunk]-stationary @ h^T[:, k-chunk]  (48 bf16
    matmuls, N=8 moving) accumulated in 3 PSUM banks (one per gate group so
    the c~/i elementwise chains overlap the o matmuls), + x^T_t via DVE,
  then ACT sigmoid/tanh on [128, 32] slices, DVE for the c/h updates.

x^T = inputs @ W_in and f^T = sigmoid(signatures @ W_f + b_f) are precomputed
per 128-step chunk with full-width matmuls (cheap), biases folded in at the
PSUM->SBUF eviction.

The T loop is a hardware For_i over chunk PAIRS (ping-pong SBUF slots inside
the body) — keeps the program ~16K instructions instead of ~60K.

Host side pre-permutes weights into gate order [c~, i, o] and pre-transposes /
pre-casts inputs to bf16, so the device never transposes anything.
"""

import numpy as np
import ml_dtypes

# Problem shapes (hardcoded per harness contract)
B, T, F = 64, 1024, 256
U = 512
SIG = 31
NCORES = 8
BL = B // NCORES  # 8 batch per core

T_CHUNK = 128
KC = U // 128        # 4 k-chunks of h/units
MC = (3 * U) // 128  # 12 gate-column chunks
FC = F // 128        # 2 k-chunks of input features

_cache = {}


def _split_excess_waits(nc, limit=1):
    """This walrus build rejects >1 sync-wait command per instruction
    ('Too many sync wait commands', CoreV2/V3 setupSyncWait). Hoist excess
    waits onto same-engine NoOp instructions inserted just before the
    offending instruction — the engine sequencer processes its queue in
    order, so the waits still complete before the instruction issues."""
    import concourse.mybir as mybir
    import bass_rust as _br

    cnt = 0
    for f in nc.m.functions:
        for b in f.blocks:
            il = b.instructions
            if not any(
                i.sync_info and i.sync_info.on_wait and len(i.sync_info.on_wait) > limit
                for i in il
            ):
                continue
            new = []
            for inst in il:
                si = inst.sync_info
                waits = list(si.on_wait) if si and si.on_wait else []
                if len(waits) > limit:
                    for w in waits[:-limit]:
                        nop = mybir.InstNoOp(name=f"wsplit_{cnt}", ins=[], outs=[])
                        cnt += 1
                        nop.engine = inst.engine
                        nop.sync_info = _br.SyncInfo(on_wait=[w], on_update=[])
                        new.append(nop)
                    si.on_wait = waits[-limit:]
                new.append(inst)
            il[:] = new
    return cnt


WREC_SCALE = 16.0  # wrec pre-scaled x16 into fp8e3m4's normal range


def _ws_for(wrec_fp8, fused):
    # fused path halves/quarters wrec on the host (sigmoid-as-tanh column
    # scales + h~=2h row scale), so a larger fp8 scale keeps values normal
    return (64.0 if fused else WREC_SCALE) if wrec_fp8 else 1.0


def _build_nc(compute_dt_name="bfloat16", wrec_dt_name="bfloat16", t_eff=T,
              shadow_prewrite=False, o_split=False, pre_spread=False,
              x_via_mm=False, diag_no_h=False, act_order_b=False,
              fused_chain=False):
    import concourse.bass as bass
    import concourse.mybir as mybir
    import concourse.tile as tile
    from contextlib import ExitStack

    fp32 = mybir.dt.float32
    cdt = getattr(mybir.dt, compute_dt_name)
    wdt = getattr(mybir.dt, wrec_dt_name)
    ws = _ws_for(wrec_dt_name.startswith("float8"), fused_chain)
    AF = mybir.ActivationFunctionType
    ALU = mybir.AluOpType
    ds = bass.ds

    assert t_eff % (2 * T_CHUNK) == 0
    nc = bass.Bass()

    # t dim padded by one body (2*T_CHUNK) so the loop's next-iteration
    # prefetch never reads out of bounds
    t_pad = t_eff + 2 * T_CHUNK
    x_in = nc.declare_dram_parameter("inputs_t", [F, BL, t_pad], cdt, isOutput=False)
    sig_in = nc.declare_dram_parameter("sig_t", [SIG, BL, t_pad], cdt, isOutput=False)
    wrec_in = nc.declare_dram_parameter("wrec", [128, KC * MC * 128], wdt, isOutput=False)
    win_in = nc.declare_dram_parameter("win", [128, FC * MC * 128], cdt, isOutput=False)
    wsig_in = nc.declare_dram_parameter("wsig", [SIG, U], cdt, isOutput=False)
    bias_g_in = nc.declare_dram_parameter("bias_g", [128, MC], fp32, isOutput=False)
    bias_f_in = nc.declare_dram_parameter("bias_f", [128, KC], fp32, isOutput=False)
    ident_in = (nc.declare_dram_parameter("ident", [128, 128], cdt, isOutput=False)
                if x_via_mm else None)
    h_out = nc.declare_dram_parameter("h_out", [128, KC * BL], fp32, isOutput=True)

    with ExitStack() as ctx:
        tc = ctx.enter_context(tile.TileContext(nc))

        const = ctx.enter_context(tc.tile_pool(name="const", bufs=1))
        state = ctx.enter_context(tc.tile_pool(name="state", bufs=1))
        data = ctx.enter_context(tc.tile_pool(name="data", bufs=1))
        work = ctx.enter_context(tc.tile_pool(name="work", bufs=6 if diag_no_h else 3))
        if fused_chain:
            psum_s = ctx.enter_context(tc.tile_pool(name="psum_s", bufs=1, space="PSUM"))
        else:
            psum_g = ctx.enter_context(tc.tile_pool(name="psum_g", bufs=2, space="PSUM"))
        psum_p = ctx.enter_context(tc.tile_pool(name="psum_p", bufs=2, space="PSUM"))

        wrec = const.tile([128, KC * MC * 128], wdt)
        nc.sync.dma_start(out=wrec[:], in_=wrec_in[:])
        win = const.tile([128, FC * MC * 128], cdt)
        nc.sync.dma_start(out=win[:], in_=win_in[:])
        wsig = const.tile([SIG, U], cdt)
        nc.sync.dma_start(out=wsig[:], in_=wsig_in[:])
        bias_g = const.tile([128, MC], fp32)
        nc.sync.dma_start(out=bias_g[:], in_=bias_g_in[:])
        bias_f = const.tile([128, KC], fp32)
        nc.sync.dma_start(out=bias_f[:], in_=bias_f_in[:])
        if x_via_mm:
            ident = const.tile([128, 128], cdt)
            nc.sync.dma_start(out=ident[:], in_=ident_in[:])

        h_bf = state.tile([128, KC * BL], cdt)      # h^T bf16, col = 8*k + b
        c_st = state.tile([128, KC * BL], fp32)     # c^T fp32
        nc.vector.memset(h_bf[:], 0.0)
        nc.vector.memset(c_st[:], 0.0)
        if diag_no_h:
            # DIAGNOSTIC ONLY (wrong results): matmuls read this constant
            # instead of h, removing the DVE->PE h dependency to measure the
            # pure LDW-stream floor.
            h_const = state.tile([128, KC * BL], cdt)
            nc.vector.memset(h_const[:], 0.01)

        W = KC * BL     # 32

        if fused_chain:
            # Fused-tanh chain: sigma(x) = (tanh(x/2)+1)/2 with the /2 folded
            # into the host-side i/o weight columns, so ALL gate activations
            # are Tanh and fuse into 2 ACT ops per step:
            #   P_ci [128, 2W] holds [c~ | i] preacts  -> ACT_a = tanh(P_ci)
            #   P_oc [128, 2W] holds [o^ | c_state]    -> ACT_b = tanh(P_oc)
            # c lives IN PSUM next to o^ so one ACT covers both; h~ = 2h =
            # (tanh(o^)+1) * tanh(c) feeds the matmuls with W_rec rows halved.
            # bank-sized (512 fp32) tiles so p_ci / p_oc / ps_pre never share
            # a PSUM bank (cross-engine bank port conflicts); only the first
            # 2W columns are used
            p_ci = psum_s.tile([128, 512], fp32, name="pci")
            p_oc = psum_s.tile([128, 512], fp32, name="poc")
            scr = work.tile([128, 2 * W], cdt, tag="scr", name="scr")
            nc.vector.memset(scr[:], 0.0)
            # warm has_written once on the matmul-accumulated regions
            nc.tensor.matmul(p_ci[:, 0:2 * W], lhsT=wrec[:, 0:128], rhs=scr[:],
                             start=True, stop=True)
            nc.tensor.matmul(p_oc[:, 0:W], lhsT=wrec[:, 0:128], rhs=scr[:, 0:W],
                             start=True, stop=True)
            nc.vector.memset(p_oc[:, W:2 * W], 0.0)  # c0 = 0

        # Warm up the 6 scan PSUM bank slots (3 gate groups x 2 bufs) with a
        # dummy start=True matmul each: this sets every element's has_written
        # bit once and we never clear it again. From then on the scan
        # pre-writes x^T into the bank via DVE and the recurrent matmuls
        # accumulate on top with start=False (a DVE write does not clear
        # has_written - only a start=True matmul does).
        # (Not needed when x is injected via identity matmul with start=True.)
        if not x_via_mm and not fused_chain:
            for warm in range(2):
                for gi in range(3):
                    pg_t = psum_g.tile([128, KC * BL], fp32, tag=f"pg{gi}", name=f"pg{gi}")
                    for jj in range(KC):
                        nc.tensor.matmul(
                            pg_t[:, jj * BL:(jj + 1) * BL],
                            lhsT=wrec[:, jj * 128:(jj + 1) * 128],
                            rhs=h_bf[:, 0:BL],
                            start=True, stop=True,
                        )

        srcv = x_in.rearrange("(k p) b t -> p k b t", p=128)

        # per-half static tile sets (ping-pong inside the For_i body)
        halves = []
        for hf in range(2):
            in_sb = data.tile([128, FC * BL * T_CHUNK], cdt, name=f"in_sb{hf}")
            sig_sb = data.tile([SIG, BL * T_CHUNK], cdt, name=f"sig_sb{hf}")
            x_sb = data.tile([128, T_CHUNK * MC * BL], cdt, name=f"x_sb{hf}")
            f_sb = data.tile([128, T_CHUNK * KC * BL], cdt, name=f"f_sb{hf}")
            halves.append((in_sb, sig_sb, x_sb, f_sb))

        NT = 512 // BL  # timesteps covered per 512-wide matmul
        W = KC * BL     # 32

        def emit_dmas(t0sc, hf):
            """Stage the half's inputs+signatures from DRAM."""
            in_sb, sig_sb, x_sb, f_sb = halves[hf]
            in_sbv = in_sb.rearrange("p (k b t) -> p k b t", k=FC, b=BL)
            for k in range(FC):
                nc.sync.dma_start(out=in_sbv[:, k, :, :],
                                  in_=srcv[:, k, :, ds(t0sc, T_CHUNK)])
            nc.sync.dma_start(out=sig_sb.rearrange("p (b t) -> p b t", b=BL),
                              in_=sig_in[:, :, ds(t0sc, T_CHUNK)])

        def pre_groups(hf):
            """Generator: one x/f precompute group (matmuls + eviction) per
            next() — lets the scan interleave these into its PE/ACT idle
            windows."""
            in_sb, sig_sb, x_sb, f_sb = halves[hf]
            in_sb4 = in_sb.rearrange("p (k b t) -> p k b t", k=FC, b=BL)
            x_sb4 = x_sb.rearrange("p (t m b) -> p m b t", m=MC, b=BL)
            f_sb4 = f_sb.rearrange("p (t u b) -> p u b t", u=KC, b=BL)
            sig_sb3 = sig_sb.rearrange("p (b t) -> p b t", b=BL)
            for j in range(MC):
                for th in range(T_CHUNK // NT):
                    ps = psum_p.tile([128, 512], fp32, tag="ps_pre", name="ps_pre")
                    for k in range(FC):
                        nc.tensor.matmul(
                            ps[:],
                            lhsT=win[:, (k * MC + j) * 128:(k * MC + j + 1) * 128],
                            rhs=in_sb4[:, k, :, th * NT:(th + 1) * NT],
                            start=(k == 0), stop=(k == FC - 1),
                        )
                        # finer-grained interleave: one ~400ns matmul per
                        # yield fits the per-step PE idle window
                        if pre_spread and k < FC - 1:
                            yield
                    dst = x_sb4[:, j, :, th * NT:(th + 1) * NT]
                    # store ws*(x + b): the scan accumulates ws*W_rec@h on top
                    # and descales at the gate activations
                    nc.scalar.activation(
                        dst, ps[:], AF.Identity, bias=bias_g[:, j:j + 1],
                        scale=ws)
                    yield
            for u in range(KC):
                for th in range(T_CHUNK // NT):
                    ps = psum_p.tile([128, 512], fp32, tag="ps_pre", name="ps_pre")
                    nc.tensor.matmul(
                        ps[:],
                        lhsT=wsig[:, u * 128:(u + 1) * 128],
                        rhs=sig_sb3[:, :, th * NT:(th + 1) * NT],
                        start=True, stop=True,
                    )
                    dst = f_sb4[:, u, :, th * NT:(th + 1) * NT]
                    nc.scalar.activation(
                        dst, ps[:], AF.Sigmoid, bias=bias_f[:, u:u + 1])
                    yield

        def scan_fused(hf, pre_iter, x_next):
            """Fused-tanh scan body: 2 ACT ops per step (tanh over [c~|i],
            tanh over [o^|c]), c state lives in PSUM beside o^. Returns the
            last step's s_b tile ([tanh(o^) | tanh(c)])."""
            in_sb, sig_sb, x_sb, f_sb = halves[hf]
            h_src = h_const if diag_no_h else h_bf
            s_b = None

            def mmf(j_list):
                for j in j_list:
                    dst = (p_ci[:, j * BL:(j + 1) * BL] if j < 8
                           else p_oc[:, (j - 8) * BL:(j - 7) * BL])
                    for k in range(KC):
                        nc.tensor.matmul(
                            dst,
                            lhsT=wrec[:, (k * MC + j) * 128:(k * MC + j + 1) * 128],
                            rhs=h_src[:, k * BL:(k + 1) * BL],
                            start=False, stop=(k == KC - 1),
                            skip_group_check=True,
                        )

            for tt in range(T_CHUNK):
                # c = f*c in place in PSUM; waits only on last step's ACT_b read
                nc.vector.scalar_tensor_tensor(
                    p_oc[:, W:2 * W], f_sb[:, tt * W:(tt + 1) * W], 1.0,
                    p_oc[:, W:2 * W], op0=ALU.mult, op1=ALU.mult)
                mmf(range(0, 8))
                s_a = work.tile([128, 2 * W], fp32, tag="s_a", name="s_a")
                nc.scalar.activation(s_a[:], p_ci[:, 0:2 * W], AF.Tanh, scale=1.0 / ws)
                tmp = work.tile([128, W], fp32, tag="tmp", name="tmp")
                # 2*i*c~ = (tanh(a_i/2)+1) * tanh(a_c)
                nc.vector.scalar_tensor_tensor(
                    tmp[:], s_a[:, W:2 * W], 1.0, s_a[:, 0:W],
                    op0=ALU.add, op1=ALU.mult)
                # c += 0.5*(2 i c~)  (state kept ws-scaled for uniform ACT_b)
                nc.vector.scalar_tensor_tensor(
                    p_oc[:, W:2 * W], tmp[:], 0.5 * ws, p_oc[:, W:2 * W],
                    op0=ALU.mult, op1=ALU.add)
                # next step's [c~|i] prewrite, off the h critical path (only
                # needs this step's ACT_a to have read the bank)
                if tt + 1 < T_CHUNK:
                    xs_n = x_sb[:, (tt + 1) * MC * BL:(tt + 2) * MC * BL]
                else:
                    xs_n = x_next[:, 0:MC * BL]
                nc.vector.tensor_copy(p_ci[:, 0:2 * W], xs_n[:, 0:2 * W])
                mmf(range(8, MC))
                s_b = work.tile([128, 2 * W], fp32, tag="s_b", name="s_b")
                nc.scalar.activation(s_b[:], p_oc[:, 0:2 * W], AF.Tanh, scale=1.0 / ws)
                # h~ = (tanh(a_o/2)+1)*tanh(c) = 2h  (W_rec rows pre-halved)
                nc.vector.scalar_tensor_tensor(
                    h_bf[:], s_b[:, 0:W], 1.0, s_b[:, W:2 * W],
                    op0=ALU.add, op1=ALU.mult)
                nc.vector.tensor_copy(p_oc[:, 0:W], xs_n[:, 2 * W:3 * W])
                if pre_iter is not None:
                    if tt % 4 == 3:
                        next(pre_iter, None)
            if pre_iter is not None:
                for _ in pre_iter:
                    pass
            return s_b

        def psum_prewrite(xs):
            """Allocate the 3 gate PSUM banks and pre-write x^T into them (the
            matmuls accumulate on top; has_written bits are permanently set,
            see warmup)."""
            pgs = []
            for gi in range(3):
                pg_t = psum_g.tile([128, W], fp32, tag=f"pg{gi}", name=f"pg{gi}")
                nc.vector.tensor_copy(pg_t[:], xs[:, gi * W:(gi + 1) * W])
                pgs.append(pg_t)
            return pgs

        def scan(hf, pre_iter, pgs, x_next):
            """pgs: this half's step-0 PSUM banks, already pre-written.
            x_next: x_sb of the half that runs after this one. Returns
            (s_o, tc, pgs) with pgs pre-written for that next half's step 0."""
            in_sb, sig_sb, x_sb, f_sb = halves[hf]
            s_o = tc_t = None

            h_src = h_const if diag_no_h else h_bf

            def mm(j_list, pgs):
                for j in j_list:
                    gi, jj = j // 4, j % 4
                    for k in range(KC):
                        nc.tensor.matmul(
                            pgs[gi][:, jj * BL:(jj + 1) * BL],
                            lhsT=wrec[:, (k * MC + j) * 128:(k * MC + j + 1) * 128],
                            rhs=h_src[:, k * BL:(k + 1) * BL],
                            start=False, stop=(k == KC - 1),
                            skip_group_check=True,
                        )

            for tt in range(T_CHUNK):
                xs = x_sb[:, tt * MC * BL:(tt + 1) * MC * BL]
                if x_via_mm:
                    # inject x^T into the banks with identity matmuls
                    # (start=True): pure PE work with NO h dependency, so the
                    # PE does it in the tail window while waiting for h, and
                    # the next step's matmuls wait only on the h semaphore.
                    pgs = []
                    for gi in range(3):
                        pg_t = psum_g.tile([128, W], fp32, tag=f"pg{gi}",
                                           name=f"pg{gi}")
                        nc.tensor.matmul(
                            pg_t[:], lhsT=ident[:], rhs=xs[:, gi * W:(gi + 1) * W],
                            start=True, stop=False, skip_group_check=True)
                        pgs.append(pg_t)
                elif not shadow_prewrite:
                    pgs = psum_prewrite(xs)
                # c = f*c can start as soon as the prior step's tanh(c) read it
                nc.vector.scalar_tensor_tensor(
                    c_st[:], f_sb[:, tt * W:(tt + 1) * W], 1.0, c_st[:],
                    op0=ALU.mult, op1=ALU.mult)
                acts = pgs
                if o_split:
                    # c~ and i matmuls; their activation chain runs under the
                    # o matmuls, which are split so sigma(o)/h for h-chunks
                    # k0/k1 complete before the last matmul -> next step's
                    # k0/k1 matmuls overlap the k2/k3 tail.
                    mm(range(0, 8), pgs)
                    s_cc = work.tile([128, W], fp32, tag="s_cc", name="s_cc")
                    nc.scalar.activation(s_cc[:], acts[0][:], AF.Tanh, scale=1.0 / ws)
                    s_i = work.tile([128, W], fp32, tag="s_i", name="s_i")
                    nc.scalar.activation(s_i[:], acts[1][:], AF.Sigmoid, scale=1.0 / ws)
                    tmp = work.tile([128, W], fp32, tag="tmp", name="tmp")
                    nc.vector.scalar_tensor_tensor(
                        tmp[:], s_i[:], 1.0, s_cc[:], op0=ALU.mult, op1=ALU.mult)
                    nc.vector.scalar_tensor_tensor(
                        c_st[:], c_st[:], 1.0, tmp[:], op0=ALU.mult, op1=ALU.add)
                    tc_t = work.tile([128, W], fp32, tag="tc", name="tc")
                    nc.scalar.activation(tc_t[:], c_st[:], AF.Tanh)
                    s_o = work.tile([128, W], fp32, tag="s_o", name="s_o")
                    HW_ = W // 2
                    mm((8, 9), pgs)
                    nc.scalar.activation(s_o[:, 0:HW_], acts[2][:, 0:HW_],
                                         AF.Sigmoid, scale=1.0 / ws)
                    nc.vector.scalar_tensor_tensor(
                        h_bf[:, 0:HW_], s_o[:, 0:HW_], 1.0, tc_t[:, 0:HW_],
                        op0=ALU.mult, op1=ALU.mult)
                    mm((10, 11), pgs)
                    nc.scalar.activation(s_o[:, HW_:W], acts[2][:, HW_:W],
                                         AF.Sigmoid, scale=1.0 / ws)
                    nc.vector.scalar_tensor_tensor(
                        h_bf[:, HW_:W], s_o[:, HW_:W], 1.0, tc_t[:, HW_:W],
                        op0=ALU.mult, op1=ALU.mult)
                else:
                    # 48 matmuls: m-outer (c~ 0-3, i 4-7, o 8-11), k-inner
                    mm(range(MC), pgs)
                    # pre-write the NEXT step's banks now, in the matmul
                    # shadow, off the h -> next-step-matmul critical path
                    if shadow_prewrite and tt + 1 < T_CHUNK:
                        pgs = psum_prewrite(
                            x_sb[:, (tt + 1) * MC * BL:(tt + 2) * MC * BL])
                    # activations straight from PSUM; sigma(o) is emitted
                    # before tanh(c) so ACT doesn't queue it behind the c chain
                    s_cc = work.tile([128, W], fp32, tag="s_cc", name="s_cc")
                    nc.scalar.activation(s_cc[:], acts[0][:], AF.Tanh, scale=1.0 / ws)
                    s_i = work.tile([128, W], fp32, tag="s_i", name="s_i")
                    nc.scalar.activation(s_i[:], acts[1][:], AF.Sigmoid, scale=1.0 / ws)
                    s_o = work.tile([128, W], fp32, tag="s_o", name="s_o")
                    if not act_order_b:
                        nc.scalar.activation(s_o[:], acts[2][:], AF.Sigmoid,
                                             scale=1.0 / ws)
                    tmp = work.tile([128, W], fp32, tag="tmp", name="tmp")
                    nc.vector.scalar_tensor_tensor(
                        tmp[:], s_i[:], 1.0, s_cc[:], op0=ALU.mult, op1=ALU.mult)
                    nc.vector.scalar_tensor_tensor(
                        c_st[:], c_st[:], 1.0, tmp[:], op0=ALU.mult, op1=ALU.add)
                    tc_t = work.tile([128, W], fp32, tag="tc", name="tc")
                    nc.scalar.activation(tc_t[:], c_st[:], AF.Tanh)
                    if act_order_b:
                        # tanh(c) ahead of sigma(o) in the ACT FIFO: the c
                        # chain isn't FIFO-gated on the o-group matmuls
                        nc.scalar.activation(s_o[:], acts[2][:], AF.Sigmoid,
                                             scale=1.0 / ws)
                    nc.vector.scalar_tensor_tensor(
                        h_bf[:], s_o[:], 1.0, tc_t[:], op0=ALU.mult, op1=ALU.mult)
                # fill the PE's h-wait idle window with precompute matmuls
                if pre_iter is not None:
                    if pre_spread:
                        if tt % 2 == 1:
                            next(pre_iter, None)
                    elif tt % 4 == 3:
                        next(pre_iter, None)
            if pre_iter is not None:
                for _ in pre_iter:
                    pass
            if shadow_prewrite:
                # next half's step-0 banks (x_sb complete: pre_iter drained)
                pgs = psum_prewrite(x_next[:, 0:MC * BL])
            return s_o, tc_t, pgs

        # prologue: half0 of the first iteration, serial
        emit_dmas(0, 0)
        for _ in pre_groups(0):
            pass

        if fused_chain:
            x0 = halves[0][2]
            nc.vector.tensor_copy(p_ci[:, 0:2 * W], x0[:, 0:2 * W])
            nc.vector.tensor_copy(p_oc[:, 0:W], x0[:, 2 * W:3 * W])
            with tc.For_i(0, t_eff, 2 * T_CHUNK) as t0:
                emit_dmas(t0 + T_CHUNK, 1)
                scan_fused(0, pre_groups(1), halves[1][2])
                emit_dmas(t0 + 2 * T_CHUNK, 0)
                s_b1 = scan_fused(1, pre_groups(0), halves[0][2])
            # h~ = 2h in fp32; the host multiplies the gathered output by 0.5
            h_f = state.tile([128, KC * BL], fp32)
            nc.vector.scalar_tensor_tensor(
                h_f[:], s_b1[:, 0:W], 1.0, s_b1[:, W:2 * W],
                op0=ALU.add, op1=ALU.mult)
            nc.sync.dma_start(out=h_out[:], in_=h_f[:])
        else:
            pgs = (psum_prewrite(halves[0][2][:, 0:MC * BL])
                   if shadow_prewrite else None)

            with tc.For_i(0, t_eff, 2 * T_CHUNK) as t0:
                emit_dmas(t0 + T_CHUNK, 1)           # this iteration's half1 data
                s_o0, tc0, pgs = scan(0, pre_groups(1), pgs, halves[1][2])
                emit_dmas(t0 + 2 * T_CHUNK, 0)       # next iteration's half0 data
                s_o1, tc1, pgs = scan(1, pre_groups(0), pgs, halves[0][2])

            # final h in fp32 from the last step's stashed (static-slot) tiles
            h_f = state.tile([128, KC * BL], fp32)
            nc.vector.scalar_tensor_tensor(
                h_f[:], s_o1[:], 1.0, tc1[:], op0=ALU.mult, op1=ALU.mult)
            nc.sync.dma_start(out=h_out[:], in_=h_f[:])

    _split_excess_waits(nc)
    return nc


def _prep_host_inputs(inputs, signatures, forget_kernel, input_kernel,
                      recurrent_kernel, bias, cdt=ml_dtypes.bfloat16, t_factor=1,
                      wrec_fp8=False, fused=False):
    """Host-side shard + permute + transpose + cast. Returns in_maps list."""
    # gate order in reference: [i, c~, o]; ours: [c~, i, o]
    perm = np.concatenate([np.arange(U, 2 * U), np.arange(0, U), np.arange(2 * U, 3 * U)])
    win_p = input_kernel[:, perm]          # [F, 3U]
    wrec_p = recurrent_kernel[:, perm]     # [U, 3U]
    b_i, b_f, b_c, b_o = np.split(bias, 4)
    bias_g = np.concatenate([b_c, b_i, b_o])  # per permuted gate col, [3U]

    if fused:
        # sigmoid-as-tanh: halve i/o preactivations (sigma(x)=(tanh(x/2)+1)/2)
        # and halve W_rec rows since the device h is h~ = 2h
        s_col = np.concatenate([np.ones(U), np.full(2 * U, 0.5)]).astype(np.float32)
        win_p = win_p * s_col[None, :]
        wrec_p = wrec_p * s_col[None, :] * 0.5
        bias_g = bias_g * s_col

    # wrec blocks: [128, (k*MC + j)*128 + c] = wrec_p[128*k + p, 128*j + c]
    wr = wrec_p.reshape(KC, 128, MC, 128).transpose(1, 0, 2, 3).reshape(128, KC * MC * 128)
    wi = win_p.reshape(FC, 128, MC, 128).transpose(1, 0, 2, 3).reshape(128, FC * MC * 128)
    bg = bias_g.reshape(MC, 128).T.copy()          # [128, MC]
    bf_ = b_f.reshape(KC, 128).T.copy()            # [128, KC]

    if wrec_fp8:
        ws = _ws_for(True, fused)
        wr = (wr * ws).astype(ml_dtypes.float8_e3m4)
        bg = bg * ws  # ACT bias is applied after the input scale
    else:
        wr = wr.astype(cdt)
    wi = wi.astype(cdt)
    wsig = forget_kernel.astype(cdt)               # [SIG, U]

    in_maps = []
    for c in range(NCORES):
        bsl = slice(c * BL, (c + 1) * BL)
        # [BL, T, F] -> [F, BL, T]
        x_t = np.ascontiguousarray(inputs[bsl].transpose(2, 0, 1)).astype(cdt)
        s_t = np.ascontiguousarray(signatures[bsl].transpose(2, 0, 1)).astype(cdt)
        if t_factor > 1:
            x_t = np.tile(x_t, (1, 1, t_factor))
            s_t = np.tile(s_t, (1, 1, t_factor))
        pad = 2 * T_CHUNK
        x_t = np.concatenate([x_t, np.zeros(x_t.shape[:2] + (pad,), x_t.dtype)], axis=2)
        s_t = np.concatenate([s_t, np.zeros(s_t.shape[:2] + (pad,), s_t.dtype)], axis=2)
        in_maps.append({
            "inputs_t": x_t, "sig_t": s_t, "wrec": wr, "win": wi,
            "wsig": wsig, "bias_g": bg.astype(np.float32),
            "bias_f": bf_.astype(np.float32),
            "ident": np.eye(128, dtype=cdt),
        })
    return in_maps


BEST_CONFIG = {"fused_chain": True}


def kernel(inputs, signatures, forget_kernel, input_kernel, recurrent_kernel,
           bias, _trace=False):
    inputs = np.asarray(inputs, dtype=np.float32)
    signatures = np.asarray(signatures, dtype=np.float32)
    forget_kernel = np.asarray(forget_kernel, dtype=np.float32)
    input_kernel = np.asarray(input_kernel, dtype=np.float32)
    recurrent_kernel = np.asarray(recurrent_kernel, dtype=np.float32)
    bias = np.asarray(bias, dtype=np.float32)

    from concourse.bass_utils import run_bass_kernel_spmd

    if "nc" not in _cache:
        _cache["nc"] = _build_nc(**BEST_CONFIG)
    nc = _cache["nc"]

    fused = BEST_CONFIG.get("fused_chain", False)
    wrec_fp8 = BEST_CONFIG.get("wrec_dt_name", "bfloat16").startswith("float8")
    in_maps = _prep_host_inputs(inputs, signatures, forget_kernel,
                                input_kernel, recurrent_kernel, bias,
                                wrec_fp8=wrec_fp8, fused=fused)
    res = run_bass_kernel_spmd(nc, in_maps, list(range(NCORES)), trace=_trace)

    out = np.empty((B, U), np.float32)
    for c in range(NCORES):
        hT = res.results[c]["h_out"]                  # [128, KC*BL]
        h = hT.reshape(128, KC, BL).transpose(2, 1, 0).reshape(BL, U)
        out[c * BL:(c + 1) * BL] = h
    if fused:
        out *= 0.5
    if _trace:
        return out, res
    return out

